# revision 4
# baseline (speedup 1.0000x reference)
"""Trainium2 Bass kernel for nn_CLUB_816043786555 (CLUB loss).

Full-input contract: kernel(**inputs) takes the complete arrays, shards the
batch dim across 8 NeuronCores, runs a Bass/Tile kernel per core, and
combines tiny per-core partial sums on the host.

Math: with mu = leaky(x@W1m+b1m)@W2m+b2m, logvar = tanh(leaky(x@W1v+b1v)@W2v+b2v),
iv = exp(-logvar), ym_d = mean_i y, y2m_d = mean_i y^2:

  loss = -0.5/N * sum_{i,d} iv*(y^2 - 2*mu*y - y2m + 2*mu*ym)
       = -0.5/N * [ P1 - 2*P2 - sum_d y2m_d*B_d + 2*sum_d ym_d*C_d ]

with per-core partials P1 = sum iv*y^2, P2 = sum iv*mu*y, C_d = sum_i iv*mu,
B_d = sum_i iv, S_d = sum_i y, T_d = sum_i y^2.  All partials are produced
on-device as fp32 accumulations; the host combine is O(128) work.

Host-side prep (dtype staging only): x/y/W are cast to fp16 on the host —
identical rounding to the previous on-device cast path, but no DRAM bounce
traffic.  b2v is negated on host so tanh's bias slot can consume it.

Schedule: groups of RG=1024 rows flow through a software pipeline where
unit g runs L1(g) matmuls interleaved (per 128-wide hidden chunk) with
L2(g-1) matmuls so the PE never stalls (its p-state ramp resets on any gap).
L2's z-head (logvar) occupies interleave slots 0-1 and the mu-head slots
2-3, so tanh can free the z PSUM early while the fused q op
((mups+b2m)*iv via AFFINE_MUL_REDUCE) frees the mu PSUM one unit later.

Engine split per group (target ~100% of PE pace on ACT/DVE, Pool under):
  ACT : leaky x ~4.5 (Prelu, bias fused) + tanh + exp(->iv fp16, accum B)
  DVE : leaky x ~3.5 (custom op, bias fused) + q(affine from PSUM, accum C)
        + p2s (q*y, accum P2) + p1s (iv*y2, accum P1)   [2x fp16 modes]
  Pool: y2 (y*y), t2s (accum T), ss (accum S)           [SBUF-only]

Precision: fp16 everywhere except PSUM accumulation, mu (stays f32 inside
the fused affine), and the f32 partial accumulators; T sums the same
rounded fp16 y^2 tile that P1 consumes (bias cancellation).
"""

import numpy as np

N_CORES = 8
N = 131072
D = 128
X_DIM = 128
H2 = 512
M = N // N_CORES          # rows per core = 16384
RG = 1024                 # rows per group
NG = M // RG              # groups per core = 16
NEG_SLOPE = 0.2

# Per-group leaky->DVE unit assignment (units indexed u = c*2 + k in emission
# order).  Even groups run 4 units on DVE, odd groups 3 (avg 3.5/4.5 split).
LEAKY_DVE_EVEN = (1, 3, 5, 7)
LEAKY_DVE_ODD = (1, 3, 5)

# B = sum(iv) accumulated for free on the ACT exp op (sums pre-rounding f32
# exp values; P1/C consume the rounded fp16 iv).  False spends a cheap 4x
# DVE tensor_scalar on an exactly-consistent B instead.
USE_EXP_ACCUM_B = False


def _leaky_on_dve(g, u):
    return u in (LEAKY_DVE_EVEN if g % 2 == 0 else LEAKY_DVE_ODD)


_leaky_op = None


def _get_leaky_op():
    """Custom DVE uop: out = max((in0 + s0) * imm2, in0 + s0) — fused
    bias-add + leaky-relu in one 1x pass straight from PSUM."""
    global _leaky_op
    if _leaky_op is not None:
        return _leaky_op
    import concourse.dve_ops as DO
    from concourse.dve_spec import C0, C2, Spec, Src0, maxx

    op = DO.DveOp(
        "LEAKY_BIAS_ANT",
        Spec(
            body=maxx((Src0 + C0) * C2, Src0 + C0),
            reference=lambda in0, in1, s0, s1, imm2: np.maximum(
                (in0.astype(np.float32) + s0) * imm2,
                in0.astype(np.float32) + s0),
        ),
        subdim=False,
        uops_sha={"v3": "28ce115f5da0f06f", "v4": ""},
    )
    DO.OPS.append(op)
    DO.CUSTOM_DVE_SPECS[op.name] = op.spec
    DO._SUB_OPCODE_FOR_NAME[op.name] = DO._CUSTOM_DVE_ROW_BASE + len(DO.OPS) - 1
    assert DO._SUB_OPCODE_FOR_NAME[op.name] < 0x20
    _leaky_op = op
    return op


_compiled = None


def _build():
    import concourse.bacc as bacc
    import concourse.tile as tile
    import concourse.mybir as mybir

    F32 = mybir.dt.float32
    F16 = mybir.dt.float16
    AF = mybir.ActivationFunctionType
    OP = mybir.AluOpType

    nc = bacc.Bacc("TRN2", target_bir_lowering=False, debug=False,
                   num_devices=N_CORES)

    x_d = nc.dram_tensor("x16", [M, X_DIM], F16, kind="ExternalInput")
    y_d = nc.dram_tensor("y16", [M, D], F16, kind="ExternalInput")
    w1_d = [nc.dram_tensor("w1m16", [X_DIM, H2], F16, kind="ExternalInput"),
            nc.dram_tensor("w1v16", [X_DIM, H2], F16, kind="ExternalInput")]
    # W2 pre-shuffled on host to [128, 4*128]: w2[p, c*128+d] = W2[c*128+p, d]
    w2_d = [nc.dram_tensor("w2m16", [128, 4 * D], F16, kind="ExternalInput"),
            nc.dram_tensor("w2v16", [128, 4 * D], F16, kind="ExternalInput")]
    # b1 host-reshaped to [128, 4]: b1r[p, c] = b1[c*128+p]
    b1_d = [nc.dram_tensor("b1m_r", [128, 4], F32, kind="ExternalInput"),
            nc.dram_tensor("b1v_r", [128, 4], F32, kind="ExternalInput")]
    b2m_d = nc.dram_tensor("b2m_r", [D, 1], F32, kind="ExternalInput")
    nb2v_d = nc.dram_tensor("nb2v_r", [D, 1], F32, kind="ExternalInput")
    out_d = nc.dram_tensor("out", [6, D, NG], F32, kind="ExternalOutput")

    leaky_op = _get_leaky_op()

    with tile.TileContext(nc) as tc:
        with (
            tc.tile_pool(name="consts", bufs=1) as consts,
            tc.tile_pool(name="tposed", bufs=2) as tposed,
            tc.tile_pool(name="hidden", bufs=2) as hidden,
            tc.tile_pool(name="l2", bufs=2) as l2pool,
            tc.tile_pool(name="junk", bufs=1) as junk,
            tc.tile_pool(name="hpsum", bufs=2, space="PSUM") as hpsum,
            tc.tile_pool(name="l2psum", bufs=1, space="PSUM") as l2psum,
        ):
            def load_group(g):
                xT = tposed.tile([X_DIM, RG], F16, tag="xT")
                yT = tposed.tile([D, RG], F16, tag="yT")
                rows = slice(g * RG, (g + 1) * RG)
                nc.sync.dma_start_transpose(xT[:], x_d[rows, :])
                nc.sync.dma_start_transpose(yT[:], y_d[rows, :])
                return xT, yT

            cur = load_group(0)

            # --- weights / biases (already fp16/laid-out on host) ---
            w1h, w2h, b1t = [], [], []
            for k in range(2):
                w1 = consts.tile([X_DIM, H2], F16, tag=f"w1h{k}")
                nc.scalar.dma_start(w1[:], w1_d[k][:])
                w1h.append(w1)
                w2 = consts.tile([128, 4, D], F16, tag=f"w2h{k}")
                nc.scalar.dma_start(
                    w2[:], w2_d[k][:].rearrange("p (c d) -> p c d", c=4))
                w2h.append(w2)
                b1 = consts.tile([128, 4], F32, tag=f"b1_{k}")
                nc.scalar.dma_start(b1[:], b1_d[k][:])
                b1t.append(b1)
            b2m = consts.tile([D, 1], F32, tag="b2m")
            nc.scalar.dma_start(b2m[:], b2m_d[:])
            nb2v = consts.tile([D, 1], F32, tag="nb2v")
            nc.scalar.dma_start(nb2v[:], nb2v_d[:])

            acc = {}
            for nm in ("P1", "P2", "C", "B", "S", "T"):
                acc[nm] = consts.tile([D, NG], F32, tag=f"acc_{nm}",
                                      name=f"acc_{nm}")

            nxt = load_group(1)

            def emit_L1_chunk(g, c, xT, hts):
                for k in range(2):
                    hp = hpsum.tile([128, RG], F32, tag="hp")
                    for s in range(2):
                        nc.tensor.matmul(hp[:, s * 512:(s + 1) * 512],
                                         w1h[k][:, c * 128:(c + 1) * 128],
                                         xT[:, s * 512:(s + 1) * 512],
                                         start=True, stop=True)
                    ht = hidden.tile([128, RG], F16, tag=f"hT{k}{c}")
                    if _leaky_on_dve(g, c * 2 + k):
                        nc.vector._custom_dve(
                            leaky_op, out=ht[:], in0=hp[:],
                            s0=b1t[k][:, c:c + 1], imm2=NEG_SLOPE)
                    else:
                        nc.scalar.activation(ht[:], hp[:], AF.Prelu,
                                             bias=b1t[k][:, c:c + 1],
                                             scale=1.0, alpha=NEG_SLOPE)
                    hts[c * 2 + k] = ht

            def emit_L2_slot(slot, prev_hts, mups, zps):
                # slots 0,1 -> z-head (k=1), slots 2,3 -> mu-head (k=0)
                k = 1 if slot < 2 else 0
                ps = zps if k == 1 else mups
                for c in ((0, 1) if slot % 2 == 0 else (2, 3)):
                    for s in range(2):
                        nc.tensor.matmul(ps[:, s * 512:(s + 1) * 512],
                                         w2h[k][:, c, :],
                                         prev_hts[c * 2 + k][:, s * 512:(s + 1) * 512],
                                         start=(c == 0), stop=(c == 3))

            def emit_tanh_exp(g, zps):
                u = l2pool.tile([D, RG], F32, tag="u")
                nc.scalar.activation(u[:], zps[:], AF.Tanh,
                                     bias=nb2v[:], scale=-1.0)
                iv = l2pool.tile([D, RG], F16, tag="iv")
                if USE_EXP_ACCUM_B:
                    nc.scalar.activation(iv[:], u[:], AF.Exp,
                                         accum_out=acc["B"][:, g:g + 1])
                else:
                    nc.scalar.activation(iv[:], u[:], AF.Exp)
                return iv

            def emit_products(g, mups, iv, yT):
                # Pool computes the full-size fp16 products (SBUF-only ops);
                # DVE sums them via tensor_scalar accum, which runs in 4x
                # mode for packed fp16 SBUF operands (~327ns a tile).
                # DVE: q = (mups + b2m) * iv  (mu never rounded), accum C
                q = l2pool.tile([D, RG], F16, tag="q")
                nc.vector.affine_mul_reduce(
                    out=q[:], accum_out=acc["C"][:, g:g + 1],
                    in0=mups[:], in1=iv[:], scale=1.0, bias=b2m[:])
                y2 = l2pool.tile([D, RG], F16, tag="y2")
                nc.gpsimd.scalar_tensor_tensor(
                    out=y2[:], in0=yT[:], scalar=1.0, in1=yT[:],
                    op0=OP.mult, op1=OP.mult)
                p2t = l2pool.tile([D, RG], F16, tag="p2t")
                nc.gpsimd.scalar_tensor_tensor(
                    out=p2t[:], in0=q[:], scalar=1.0, in1=yT[:],
                    op0=OP.mult, op1=OP.mult)
                p1t = l2pool.tile([D, RG], F16, tag="p1t")
                nc.gpsimd.scalar_tensor_tensor(
                    out=p1t[:], in0=iv[:], scalar=1.0, in1=y2[:],
                    op0=OP.mult, op1=OP.mult)

                def dve_sum(src, nm):
                    j = junk.tile([D, RG], F16, tag=f"j{nm}")
                    nc.vector.tensor_scalar(
                        out=j[:], in0=src[:], scalar1=1.0, scalar2=None,
                        op0=OP.mult, op1=OP.add,
                        accum_out=acc[nm][:, g:g + 1])

                dve_sum(yT, "S")
                dve_sum(y2, "T")   # same rounded fp16 y2 that P1 consumes
                if not USE_EXP_ACCUM_B:
                    dve_sum(iv, "B")
                dve_sum(p2t, "P2")
                dve_sum(p1t, "P1")

            prev_hts = None
            prev_yT = None
            for g in range(NG):
                xT, yT = cur
                if prev_hts is not None:
                    mups = l2psum.tile([D, RG], F32, tag="mups")
                    zps = l2psum.tile([D, RG], F32, tag="zps")
                hts = {}
                for c in range(4):
                    emit_L1_chunk(g, c, xT, hts)
                    if prev_hts is not None:
                        emit_L2_slot(c, prev_hts, mups, zps)
                        if c == 1:
                            iv = emit_tanh_exp(g - 1, zps)
                if prev_hts is not None:
                    emit_products(g - 1, mups, iv, prev_yT)
                prev_hts = hts
                prev_yT = yT
                cur = nxt
                nxt = load_group(g + 2) if g + 2 < NG else None

            # drain: L2 + post-ops for the final group
            mups = l2psum.tile([D, RG], F32, tag="mups")
            zps = l2psum.tile([D, RG], F32, tag="zps")
            for slot in range(4):
                emit_L2_slot(slot, prev_hts, mups, zps)
                if slot == 1:
                    iv = emit_tanh_exp(NG - 1, zps)
            emit_products(NG - 1, mups, iv, prev_yT)

            for i, nm in enumerate(("P1", "P2", "C", "B", "S", "T")):
                nc.sync.dma_start(out_d[i], acc[nm][:])

    nc.compile()
    return nc


def _get_compiled():
    global _compiled
    if _compiled is None:
        _compiled = _build()
    return _compiled


def make_in_maps(x_samples, y_samples, W1m, b1m, W2m, b2m, W1v, b1v, W2v, b2v):
    """Host-side staging: shard x/y over cores, cast to fp16, lay out weights."""
    f16 = np.float16
    f32 = np.float32

    def w2_shuffle(W2):
        return np.ascontiguousarray(
            np.asarray(W2, f32).reshape(4, 128, D).transpose(1, 0, 2)
            .reshape(128, 4 * D).astype(f16))

    shared = {
        "w1m16": np.ascontiguousarray(np.asarray(W1m, f32).astype(f16)),
        "w1v16": np.ascontiguousarray(np.asarray(W1v, f32).astype(f16)),
        "w2m16": w2_shuffle(W2m),
        "w2v16": w2_shuffle(W2v),
        "b1m_r": np.ascontiguousarray(np.asarray(b1m, f32).reshape(4, 128).T),
        "b1v_r": np.ascontiguousarray(np.asarray(b1v, f32).reshape(4, 128).T),
        "b2m_r": np.ascontiguousarray(np.asarray(b2m, f32).reshape(D, 1)),
        "nb2v_r": np.ascontiguousarray(-np.asarray(b2v, f32).reshape(D, 1)),
    }
    xs = np.asarray(x_samples, f32).astype(f16)
    ys = np.asarray(y_samples, f32).astype(f16)
    in_maps = []
    for i in range(N_CORES):
        sl = slice(i * M, (i + 1) * M)
        m = {"x16": np.ascontiguousarray(xs[sl]),
             "y16": np.ascontiguousarray(ys[sl])}
        m.update(shared)
        in_maps.append(m)
    return in_maps


def kernel(x_samples, y_samples, W1m, b1m, W2m, b2m, W1v, b1v, W2v, b2v):
    from concourse.bass_utils import run_bass_kernel_spmd

    nc = _get_compiled()
    in_maps = make_in_maps(x_samples, y_samples, W1m, b1m, W2m, b2m,
                           W1v, b1v, W2v, b2v)
    res = run_bass_kernel_spmd(nc, in_maps, list(range(N_CORES)))
    return combine([r["out"] for r in res.results])


def combine(outs):
    """Host-side gather: sum per-core [6, 128, NG] partials and finish the loss."""
    tot = np.sum([o.astype(np.float64) for o in outs], axis=(0, 3))
    P1, P2, C, B, S, T = tot
    ym = S / N
    y2m = T / N
    total = P1.sum() - 2.0 * P2.sum() - (y2m * B).sum() + 2.0 * (ym * C).sum()
    return np.float32(-0.5 * total / N)


# revision 6
# speedup vs baseline: 1.0527x; 1.0527x over previous
"""Trainium2 Bass kernel for nn_CLUB_816043786555 (CLUB loss).

Full-input contract: kernel(**inputs) takes the complete arrays, shards the
batch dim across 8 NeuronCores, runs a Bass/Tile kernel per core, and
combines tiny per-core partial sums on the host.

Math: with mu = leaky(x@W1m+b1m)@W2m+b2m, logvar = tanh(leaky(x@W1v+b1v)@W2v+b2v),
iv = exp(-logvar), ym_d = mean_i y, y2m_d = mean_i y^2:

  loss = -0.5/N * sum_{i,d} iv*(y^2 - 2*mu*y - y2m + 2*mu*ym)
       = -0.5/N * [ P1 - 2*P2 - sum_d y2m_d*B_d + 2*sum_d ym_d*C_d ]

with per-core partials P1 = sum iv*y^2, P2 = sum iv*mu*y, C_d = sum_i iv*mu,
B_d = sum_i iv, S_d = sum_i y, T_d = sum_i y^2.  All partials are produced
on-device as fp32 accumulations; the host combine is O(128) work.

Host-side prep (dtype/layout staging only): x/y/W cast to fp16 (identical
rounding to the previous on-device cast path, no DRAM bounce), weights packed
into a single [128,2048] fp16 tensor and biases into [128,10] f32 so startup
is 2 DMAs, b2v negated for tanh's bias slot.

Schedule: groups of RG=1024 rows flow through a 3-deep software pipeline.
Unit g emits:
  - transpose-loads for group g+1
  - product stage for group g-2 (all inputs are then a full unit old, so no
    engine ever head-blocks its in-order queue on a cross-engine dependency)
  - L1(g) matmuls interleaved per 128-wide hidden chunk with L2(g-1)
    matmuls so the PE never gaps (its p-state ramp resets on any gap);
    L2's z-head occupies interleave slots 0-1 and the mu-head slots 2-3
  - tanh/exp for group g-1 (emitted mid-unit, freeing the z PSUM early;
    the mu PSUM is freed by q(g-1) at the start of unit g+1)

Engine split per group (ACT/DVE balanced ~96% of PE pace, Pool ~89%):
  ACT : leaky x ~4.3 (Prelu, bias fused) + tanh + exp (->iv fp16, accum B)
  DVE : leaky x ~3.7 (custom op from PSUM) + q = (mups+b2m)*iv (fused
        AFFINE_MUL_REDUCE, accum C) + 4x-mode tensor_scalar sums of T/P1/P2
  Pool: fp16 products y2=y*y, p1t=iv*y2, p2t=q*y, and the S sum (SBUF-only)

Precision: fp16 tensors everywhere except PSUM, mu (f32 inside the fused
affine), u=tanh (f32), and the f32 partial accumulators; T sums the same
rounded fp16 y^2 tile that P1's product consumes (bias cancellation).
"""

import numpy as np

N_CORES = 8
N = 131072
D = 128
X_DIM = 128
H2 = 512
M = N // N_CORES          # rows per core = 16384
RG = 1024                 # rows per group
NG = M // RG              # groups per core = 16
NEG_SLOPE = 0.2

# Per-group leaky->DVE unit assignment (units indexed u = c*2 + k in emission
# order).  One group in three runs 3 units on DVE, the rest 4 (avg 4.33 ACT /
# 3.67 DVE), balancing ACT and DVE at ~96% of PE pace.
LEAKY_DVE_SPARSE = (1, 3, 5)
LEAKY_DVE_DENSE = (1, 3, 5, 7)

# B = sum(iv) accumulated for free on the ACT exp op (sums pre-rounding f32
# exp values; P1/C consume the rounded fp16 iv).  False spends a cheap 4x
# DVE tensor_scalar on an exactly-consistent B instead.
USE_EXP_ACCUM_B = True


def _leaky_on_dve(g, u):
    return u in (LEAKY_DVE_SPARSE if g % 3 == 0 else LEAKY_DVE_DENSE)


_leaky_op = None


def _get_leaky_op():
    """Custom DVE uop: out = max((in0 + s0) * imm2, in0 + s0) — fused
    bias-add + leaky-relu in one 1x pass straight from PSUM."""
    global _leaky_op
    if _leaky_op is not None:
        return _leaky_op
    import concourse.dve_ops as DO
    from concourse.dve_spec import C0, C2, Spec, Src0, maxx

    op = DO.DveOp(
        "LEAKY_BIAS_ANT",
        Spec(
            body=maxx((Src0 + C0) * C2, Src0 + C0),
            reference=lambda in0, in1, s0, s1, imm2: np.maximum(
                (in0.astype(np.float32) + s0) * imm2,
                in0.astype(np.float32) + s0),
        ),
        subdim=False,
        uops_sha={"v3": "28ce115f5da0f06f", "v4": ""},
    )
    DO.OPS.append(op)
    DO.CUSTOM_DVE_SPECS[op.name] = op.spec
    DO._SUB_OPCODE_FOR_NAME[op.name] = DO._CUSTOM_DVE_ROW_BASE + len(DO.OPS) - 1
    assert DO._SUB_OPCODE_FOR_NAME[op.name] < 0x20
    _leaky_op = op
    return op


_compiled = None


def _build():
    import concourse.bacc as bacc
    import concourse.tile as tile
    import concourse.mybir as mybir

    F32 = mybir.dt.float32
    F16 = mybir.dt.float16
    AF = mybir.ActivationFunctionType
    OP = mybir.AluOpType

    nc = bacc.Bacc("TRN2", target_bir_lowering=False, debug=False,
                   num_devices=N_CORES)

    x_d = nc.dram_tensor("x16", [M, X_DIM], F16, kind="ExternalInput")
    y_d = nc.dram_tensor("y16", [M, D], F16, kind="ExternalInput")
    # wpack cols: [0:512) W1m, [512:1024) W1v, [1024:1536) W2m', [1536:2048) W2v'
    # where W2'[p, c*128+d] = W2[c*128+p, d]
    wpack_d = nc.dram_tensor("wpack16", [128, 2048], F16, kind="ExternalInput")
    # bpack cols: [0:4) b1m', [4:8) b1v' (b1'[p,c] = b1[c*128+p]), [8] b2m,
    # [9] -b2v
    bpack_d = nc.dram_tensor("bpack32", [128, 10], F32, kind="ExternalInput")
    out_d = nc.dram_tensor("out", [6, D, NG], F32, kind="ExternalOutput")

    leaky_op = _get_leaky_op()

    with tile.TileContext(nc) as tc:
        with (
            tc.tile_pool(name="consts", bufs=1) as consts,
            tc.tile_pool(name="xtp", bufs=2) as xtp,
            tc.tile_pool(name="ytp", bufs=4) as ytp,
            tc.tile_pool(name="hidden", bufs=2) as hidden,
            tc.tile_pool(name="l2", bufs=2) as l2pool,
            tc.tile_pool(name="junk", bufs=1) as junk,
            tc.tile_pool(name="hpsum", bufs=2, space="PSUM") as hpsum,
            tc.tile_pool(name="l2psum", bufs=1, space="PSUM") as l2psum,
        ):
            # --- startup: 2 packed const DMAs, then transposes, all on SP ---
            wp = consts.tile([128, 2048], F16, tag="wp")
            nc.sync.dma_start(wp[:], wpack_d[:])
            bp = consts.tile([128, 10], F32, tag="bp")
            nc.sync.dma_start(bp[:], bpack_d[:])

            def w1(k, c):
                return wp[:, k * 512 + c * 128:k * 512 + (c + 1) * 128]

            def w2(k, c):
                return wp[:, 1024 + k * 512 + c * 128:1024 + k * 512 + (c + 1) * 128]

            def b1(k, c):
                return bp[:, k * 4 + c:k * 4 + c + 1]

            b2m = bp[:, 8:9]
            nb2v = bp[:, 9:10]

            def load_group(g):
                xT = xtp.tile([X_DIM, RG], F16, tag="xT")
                yT = ytp.tile([D, RG], F16, tag="yT")
                rows = slice(g * RG, (g + 1) * RG)
                nc.sync.dma_start_transpose(xT[:], x_d[rows, :])
                nc.sync.dma_start_transpose(yT[:], y_d[rows, :])
                return xT, yT

            acc = {}
            for nm in ("P1", "P2", "C", "B", "S", "T"):
                acc[nm] = consts.tile([D, NG], F32, tag=f"acc_{nm}",
                                      name=f"acc_{nm}")

            loads = [load_group(0), load_group(1)]
            hts_hist = {}     # g -> dict u -> ht tile
            iv_hist = {}      # g -> iv tile
            q_hist = {}       # g -> q tile
            mups_hist = {}    # g -> mups psum tile

            def emit_L1_chunk(g, c, xT):
                for k in range(2):
                    hp = hpsum.tile([128, RG], F32, tag="hp")
                    for s in range(2):
                        nc.tensor.matmul(hp[:, s * 512:(s + 1) * 512],
                                         w1(k, c),
                                         xT[:, s * 512:(s + 1) * 512],
                                         start=True, stop=True)
                    ht = hidden.tile([128, RG], F16, tag=f"hT{k}{c}")
                    if _leaky_on_dve(g, c * 2 + k):
                        nc.vector._custom_dve(
                            leaky_op, out=ht[:], in0=hp[:],
                            s0=b1(k, c), imm2=NEG_SLOPE)
                    else:
                        nc.scalar.activation(ht[:], hp[:], AF.Prelu,
                                             bias=b1(k, c),
                                             scale=1.0, alpha=NEG_SLOPE)
                    hts_hist[g][c * 2 + k] = ht

            def emit_L2_slot(slot, hts, mups, zps):
                # slots 0,1 -> z-head (k=1), slots 2,3 -> mu-head (k=0)
                k = 1 if slot < 2 else 0
                ps = zps if k == 1 else mups
                for c in ((0, 1) if slot % 2 == 0 else (2, 3)):
                    for s in range(2):
                        nc.tensor.matmul(ps[:, s * 512:(s + 1) * 512],
                                         w2(k, c),
                                         hts[c * 2 + k][:, s * 512:(s + 1) * 512],
                                         start=(c == 0), stop=(c == 3))

            def emit_tanh_exp(g, zps):
                u = l2pool.tile([D, RG], F32, tag="u")
                nc.scalar.activation(u[:], zps[:], AF.Tanh,
                                     bias=nb2v, scale=-1.0)
                iv = l2pool.tile([D, RG], F16, tag="iv")
                if USE_EXP_ACCUM_B:
                    nc.scalar.activation(iv[:], u[:], AF.Exp,
                                         accum_out=acc["B"][:, g:g + 1])
                else:
                    nc.scalar.activation(iv[:], u[:], AF.Exp)
                iv_hist[g] = iv

            def emit_products_head(g):
                """Start-of-unit portion: q frees the mu PSUM; Pool products.
                All inputs are >= one full unit old."""
                iv, yT, mups = iv_hist[g], loads[g][1], mups_hist.pop(g)
                q = l2pool.tile([D, RG], F16, tag="q")
                nc.vector.affine_mul_reduce(
                    out=q[:], accum_out=acc["C"][:, g:g + 1],
                    in0=mups[:], in1=iv[:], scale=1.0, bias=b2m)
                q_hist[g] = q
                y2 = l2pool.tile([D, RG], F16, tag="y2")
                nc.gpsimd.scalar_tensor_tensor(
                    out=y2[:], in0=yT[:], scalar=1.0, in1=yT[:],
                    op0=OP.mult, op1=OP.mult)
                p1t = l2pool.tile([D, RG], F16, tag="p1t")
                nc.gpsimd.scalar_tensor_tensor(
                    out=p1t[:], in0=iv[:], scalar=1.0, in1=y2[:],
                    op0=OP.mult, op1=OP.mult)
                p2t = l2pool.tile([D, RG], F16, tag="p2t")
                nc.gpsimd.scalar_tensor_tensor(
                    out=p2t[:], in0=q[:], scalar=1.0, in1=yT[:],
                    op0=OP.mult, op1=OP.mult)
                jS = junk.tile([D, RG], F16, tag="jS")
                nc.gpsimd.tensor_scalar(
                    out=jS[:], in0=yT[:], scalar1=1.0, scalar2=None,
                    op0=OP.mult, op1=OP.add,
                    accum_out=acc["S"][:, g:g + 1])
                if not USE_EXP_ACCUM_B:
                    jB = junk.tile([D, RG], F16, tag="jB")
                    nc.vector.tensor_scalar(
                        out=jB[:], in0=iv[:], scalar1=1.0, scalar2=None,
                        op0=OP.mult, op1=OP.add,
                        accum_out=acc["B"][:, g:g + 1])
                return y2, p1t, p2t

            def emit_sums_tail(g, y2, p1t, p2t):
                """End-of-unit 4x DVE sums over the Pool products."""
                for src, nm in ((y2, "T"), (p1t, "P1"), (p2t, "P2")):
                    j = junk.tile([D, RG], F16, tag=f"j{nm}")
                    nc.vector.tensor_scalar(
                        out=j[:], in0=src[:], scalar1=1.0, scalar2=None,
                        op0=OP.mult, op1=OP.add,
                        accum_out=acc[nm][:, g:g + 1])

            prods = None
            for g in range(NG):
                if g + 2 < NG:
                    loads.append(load_group(g + 2))
                if g >= 2:
                    prods = emit_products_head(g - 2)
                hts_hist[g] = {}
                if g >= 1:
                    mups = l2psum.tile([D, RG], F32, tag="mups")
                    zps = l2psum.tile([D, RG], F32, tag="zps")
                    mups_hist[g - 1] = mups
                for c in range(4):
                    emit_L1_chunk(g, c, loads[g][0])
                    if g >= 1:
                        emit_L2_slot(c, hts_hist[g - 1], mups, zps)
                        if c == 1:
                            emit_tanh_exp(g - 1, zps)
                if g >= 2:
                    emit_sums_tail(g - 2, *prods)
                    del hts_hist[g - 2]

            # drain unit NG: L2(NG-1) + tanh/exp(NG-1) + products(NG-2)
            prods = emit_products_head(NG - 2)
            mups = l2psum.tile([D, RG], F32, tag="mups")
            zps = l2psum.tile([D, RG], F32, tag="zps")
            mups_hist[NG - 1] = mups
            for slot in range(4):
                emit_L2_slot(slot, hts_hist[NG - 1], mups, zps)
                if slot == 1:
                    emit_tanh_exp(NG - 1, zps)
            emit_sums_tail(NG - 2, *prods)

            # drain unit NG+1: products(NG-1), then outputs
            prods = emit_products_head(NG - 1)
            emit_sums_tail(NG - 1, *prods)

            for i, nm in enumerate(("C", "B", "S", "T", "P1", "P2")):
                nc.sync.dma_start(out_d[i], acc[nm][:])

    nc.compile()
    return nc


def _get_compiled():
    global _compiled
    if _compiled is None:
        _compiled = _build()
    return _compiled


def make_in_maps(x_samples, y_samples, W1m, b1m, W2m, b2m, W1v, b1v, W2v, b2v):
    """Host-side staging: shard x/y over cores, cast to fp16, pack weights."""
    f16 = np.float16
    f32 = np.float32

    def w2_shuffle(W2):
        return (np.asarray(W2, f32).reshape(4, 128, D).transpose(1, 0, 2)
                .reshape(128, 4 * D))

    wpack = np.concatenate([
        np.asarray(W1m, f32), np.asarray(W1v, f32),
        w2_shuffle(W2m), w2_shuffle(W2v)], axis=1).astype(f16)
    bpack = np.concatenate([
        np.asarray(b1m, f32).reshape(4, 128).T,
        np.asarray(b1v, f32).reshape(4, 128).T,
        np.asarray(b2m, f32).reshape(128, 1),
        -np.asarray(b2v, f32).reshape(128, 1)], axis=1)
    shared = {
        "wpack16": np.ascontiguousarray(wpack),
        "bpack32": np.ascontiguousarray(bpack.astype(f32)),
    }
    xs = np.asarray(x_samples, f32).astype(f16)
    ys = np.asarray(y_samples, f32).astype(f16)
    in_maps = []
    for i in range(N_CORES):
        sl = slice(i * M, (i + 1) * M)
        m = {"x16": np.ascontiguousarray(xs[sl]),
             "y16": np.ascontiguousarray(ys[sl])}
        m.update(shared)
        in_maps.append(m)
    return in_maps


def kernel(x_samples, y_samples, W1m, b1m, W2m, b2m, W1v, b1v, W2v, b2v):
    from concourse.bass_utils import run_bass_kernel_spmd

    nc = _get_compiled()
    in_maps = make_in_maps(x_samples, y_samples, W1m, b1m, W2m, b2m,
                           W1v, b1v, W2v, b2v)
    res = run_bass_kernel_spmd(nc, in_maps, list(range(N_CORES)))
    return combine([r["out"] for r in res.results])


def combine(outs):
    """Host-side gather: sum per-core [6, 128, NG] partials and finish the loss."""
    tot = np.sum([o.astype(np.float64) for o in outs], axis=(0, 3))
    C, B, S, T, P1, P2 = tot
    ym = S / N
    y2m = T / N
    total = P1.sum() - 2.0 * P2.sum() - (y2m * B).sum() + 2.0 * (ym * C).sum()
    return np.float32(-0.5 * total / N)
